# revision 29
# baseline (speedup 1.0000x reference)
"""Trainium2 Bass kernel for GNN mean aggregation (nn_AggrGSMean).

Computes, for t in {0,1}:
    out_t[b, v, :] = segment_sum(features_t over edges with dest v) / degree[b, v, t]
where degree[b, v, t] = max(count(adjacency[b, v, t, :] >= 0), 1).

Strategy (graph-partition sharding):
- Host: partition edges by destination-vertex range across 8 cores, sort each
  core's edges by destination.  Vertices form 128-wide blocks (the PSUM
  partition dim), each split into four 32-vertex windows.  Full 128-edge
  tiles are window-pure (32-column one-hot via PE tile_position); each
  window's leftover edges pool into block-level "tail" tiles with 128-wide
  one-hots, so roundup waste is per-block not per-window.  Windows are
  sorted within a block and blocks sorted by tile count; a per-table static
  profile (max over cores at each rank) serves all cores.  Features ship as
  single bf16 (the 2e-2 rel-err budget gives ~10x margin); each edge carries
  a bf16 one-hot position (vertex-in-window for full tiles, vertex-in-block
  for tails; both exact in bf16).
- The HBM stream is partition-major per 7-slot group, so each DMA moves
  [128, ~16KB] with fully contiguous lines.  Table 0 streams on the
  sync-engine HWDGE queue, table 1 on the scalar-engine queue.
- Device per (slot, table): GPSIMD builds the tail one-hots, DVE the window
  one-hots (is_equal vs tiled iota); matmuls accumulate into one [128, 64]
  PSUM tile (tails full-width first with start=True, then 32-column windows
  at tile_position (0, 32j)).  Degree comes from an int8 adjacency slice
  (is_ge + reduce + recip on DVE); the mean division rides a ScalarE copy
  into a grouped bf16 output tile.
"""

import sys

if "/opt/trn_rl_repo" not in sys.path:
    sys.path.insert(0, "/opt/trn_rl_repo")

import ml_dtypes
import numpy as np

# Problem constants (hardcoded per contract)
B, V, T, N, F, M = 1, 100000, 2, 32, 64, 1600000
NCORES = 8
BLK = 128            # edges per tile (matmul contraction)
BLK_V = 128          # vertices per block (PSUM partition dim)
WIN = 32             # vertices per one-hot window (stationary columns)
NW = BLK_V // WIN    # windows per block
GRP = 7              # slots per DMA group


class Cfg:
    def __init__(self, v=V, ncores=NCORES):
        self.V = v
        self.NCORES = ncores
        self.VLOC = v // ncores
        nblk = (self.VLOC + BLK_V - 1) // BLK_V
        self.NBLK = ((nblk + GRP - 1) // GRP) * GRP
        self.NG = self.NBLK // GRP
        self.VPAD = self.NBLK * BLK_V


_DEFAULT_CFG = Cfg()
_NC_CACHE = {}


def _layout(prof5):
    """Derived layout from a [NBLK, 5] profile (col 0 tails, 1..4 windows)."""
    prof5 = np.asarray(prof5, dtype=np.int64)
    nblk = prof5.shape[0]
    tails = prof5[:, 0]
    wins = prof5[:, 1:]
    prof_t = prof5.sum(axis=1)                        # feat tiles per slot
    fo = np.zeros((nblk, NW), dtype=np.int64)         # flat tile offset of window rank
    fo[:, 0] = tails
    fo[:, 1:] = tails[:, None] + np.cumsum(wins[:, :-1], axis=1)
    ohc = tails * BLK_V + wins.sum(axis=1) * WIN      # one-hot cols per slot
    iw = prof_t                                       # idx cols per slot (bf16 pos)
    ng = nblk // GRP
    pt = prof_t.reshape(ng, GRP)
    iwg = iw.reshape(ng, GRP)
    fb = np.zeros((ng, GRP), dtype=np.int64)          # feat word offset in group line
    fb[:, 1:] = np.cumsum(pt[:, :-1] * F, axis=1)
    fw = (pt * F).sum(axis=1)                         # feat words per group line
    ib = np.zeros((ng, GRP), dtype=np.int64)
    ib[:, 1:] = np.cumsum(iwg[:, :-1], axis=1)
    lw = fw + iwg.sum(axis=1)                         # words per partition line
    gb = np.zeros(ng + 1, dtype=np.int64)
    gb[1:] = np.cumsum(lw * BLK)
    return dict(prof5=prof5, tails=tails, wins=wins, prof_t=prof_t, fo=fo,
                ohc=ohc, iw=iw, fb=fb, fw=fw, ib=ib, lw=lw, gb=gb,
                tmax_t=int(tails.max()), tmax_w=int(wins.sum(axis=1).max()),
                ohmax=int(ohc.max()))


def build_device_program(profiles, cfg=_DEFAULT_CFG):
    """Build + compile the per-core Bass program for per-table [NBLK,5] profiles."""
    from contextlib import ExitStack

    import concourse.tile as tile
    from concourse import bacc, mybir

    f32 = mybir.dt.float32
    bf16 = mybir.dt.bfloat16
    i16 = mybir.dt.int16
    i8 = mybir.dt.int8

    lays = [_layout(p) for p in profiles]
    NBLK, NG = cfg.NBLK, cfg.NG
    tmax_t = max(l["tmax_t"] for l in lays)
    tmax_w = max(l["tmax_w"] for l in lays)
    ohmax = max(l["ohmax"] for l in lays)
    lwmax = max(int(l["lw"].max()) for l in lays)

    nc = bacc.Bacc("TRN2", target_bir_lowering=False, debug=False)
    feat_d = [
        nc.dram_tensor(f"feat{t}", [int(lays[t]["gb"][-1])], bf16,
                       kind="ExternalInput").ap()
        for t in range(T)
    ]
    adj_d = nc.dram_tensor("adj", [NG, BLK, GRP * T * N], i8, kind="ExternalInput").ap()
    iota_d = nc.dram_tensor("iota", [BLK, tmax_t * BLK_V + tmax_w * WIN], bf16,
                            kind="ExternalInput").ap()
    out_d = nc.dram_tensor("out", [NG, BLK, GRP * T * F], bf16,
                           kind="ExternalOutput").ap()

    with tile.TileContext(nc) as tc, ExitStack() as ctx:
        const = ctx.enter_context(tc.tile_pool(name="const", bufs=1))
        featp = ctx.enter_context(tc.tile_pool(name="featp", bufs=4))
        adjp = ctx.enter_context(tc.tile_pool(name="adjp", bufs=3))
        degp = ctx.enter_context(tc.tile_pool(name="degp", bufs=3))
        ohp = ctx.enter_context(tc.tile_pool(name="ohp", bufs=6))
        outp = ctx.enter_context(tc.tile_pool(name="outp", bufs=3))
        psump = ctx.enter_context(tc.tile_pool(name="psum", bufs=8, space="PSUM"))

        iota = const.tile([BLK, tmax_t * BLK_V + tmax_w * WIN], bf16)
        nc.sync.dma_start(out=iota[:], in_=iota_d[:])
        wio = tmax_t * BLK_V  # window iota offset

        for g in range(NG):
            adj_t = adjp.tile([BLK, GRP * T * N], i8)
            nc.sync.dma_start(out=adj_t[:], in_=adj_d[g])
            val = degp.tile([BLK, GRP * T * N], bf16, tag="val")
            nc.vector.tensor_scalar(
                val[:], adj_t[:], 0, None, op0=mybir.AluOpType.is_ge
            )
            deg = degp.tile([BLK, GRP * T], f32, tag="deg")
            nc.vector.tensor_reduce(
                deg[:],
                val[:].rearrange("p (x n) -> p x n", n=N),
                axis=mybir.AxisListType.X,
                op=mybir.AluOpType.add,
            )
            rec = degp.tile([BLK, GRP * T], f32, tag="rec")
            nc.vector.tensor_scalar(
                deg[:], deg[:], 1.0, None, op0=mybir.AluOpType.max
            )
            nc.vector.reciprocal(rec[:], deg[:])

            feats = []
            for t in range(T):
                lay = lays[t]
                ft = featp.tile([BLK, lwmax], bf16, tag=f"feat{t}")
                src = feat_d[t][int(lay["gb"][g]) : int(lay["gb"][g + 1])].rearrange(
                    "(p w) -> p w", w=int(lay["lw"][g])
                )
                eng = nc.sync if t == 0 else nc.scalar
                eng.dma_start(out=ft[:, : int(lay["lw"][g])], in_=src)
                feats.append(ft)

            out_t = outp.tile([BLK, GRP * T * F], bf16)
            for q in range(GRP):
                s = g * GRP + q
                for t in range(T):
                    lay = lays[t]
                    ft = feats[t]
                    nt = int(lay["tails"][s])
                    nwt = int(lay["wins"][s].sum())
                    ts = nt + nwt
                    fbase = int(lay["fb"][g, q])
                    io = int(lay["fw"][g]) + int(lay["ib"][g, q])
                    oh = ohp.tile([BLK, ohmax], bf16, tag="oh")
                    # tail one-hots (128-wide) on DVE
                    nc.vector.tensor_tensor(
                        oh[:, : nt * BLK_V].rearrange("p (i v) -> p i v", v=BLK_V),
                        iota[:, : nt * BLK_V].rearrange("p (i v) -> p i v", v=BLK_V),
                        ft[:, io : io + nt].unsqueeze(2)
                        .broadcast_to([BLK, nt, BLK_V]),
                        op=mybir.AluOpType.is_equal,
                    )
                    # window one-hots (32-wide) on DVE
                    if nwt > 0:
                        nc.vector.tensor_tensor(
                            oh[:, nt * BLK_V : nt * BLK_V + nwt * WIN].rearrange(
                                "p (i v) -> p i v", v=WIN
                            ),
                            iota[:, wio : wio + nwt * WIN].rearrange(
                                "p (i v) -> p i v", v=WIN
                            ),
                            ft[:, io + nt : io + ts].unsqueeze(2)
                            .broadcast_to([BLK, nwt, WIN]),
                            op=mybir.AluOpType.is_equal,
                        )
                    ps = psump.tile([BLK, F], f32)
                    for i in range(nt):
                        nc.tensor.matmul(
                            ps[:],
                            lhsT=oh[:, i * BLK_V : (i + 1) * BLK_V],
                            rhs=ft[:, fbase + i * F : fbase + (i + 1) * F],
                            start=(i == 0),
                            stop=(i == ts - 1),
                        )
                    wco = nt * BLK_V
                    for j in range(NW):
                        nj = int(lay["wins"][s, j])
                        for il in range(nj):
                            i = int(lay["fo"][s, j]) + il
                            c = wco + (i - nt) * WIN
                            nc.tensor.matmul(
                                ps[j * WIN : (j + 1) * WIN, :],
                                lhsT=oh[:, c : c + WIN],
                                rhs=ft[:, fbase + i * F : fbase + (i + 1) * F],
                                start=False,
                                stop=(i == ts - 1),
                                tile_position=(0, j * WIN),
                            )
                    nc.scalar.mul(
                        out_t[:, (q * T + t) * F : (q * T + t + 1) * F],
                        ps[:],
                        rec[:, q * T + t : q * T + t + 1],
                    )
            nc.sync.dma_start(out=out_d[g], in_=out_t[:])

    nc.compile()
    return nc


def shard_table(indices, cfg=_DEFAULT_CFG):
    """Sort edges by destination, partition by core, build per-core schedule."""
    v = np.ascontiguousarray(indices[:, 1])
    order = np.argsort(v, kind="stable")
    vs = v[order]
    bounds = np.searchsorted(vs, np.arange(cfg.NCORES + 1) * cfg.VLOC)
    per_core = []
    for c in range(cfg.NCORES):
        lo, hi = bounds[c], bounds[c + 1]
        idx_e = order[lo:hi]
        vloc = vs[lo:hi].astype(np.int64) - c * cfg.VLOC
        bw = vloc >> 5                       # block*4 + window
        u = vloc & 31
        cnt = np.bincount(bw, minlength=cfg.NBLK * NW).reshape(cfg.NBLK, NW)
        full = cnt >> 7
        rem = cnt & 127
        tails = (rem.sum(axis=1) + BLK - 1) // BLK
        win_perm = np.argsort(-full, axis=1, kind="stable")    # [NBLK, 4] rank->win
        blk_tot = tails + full.sum(axis=1)
        blk_perm = np.argsort(-blk_tot, kind="stable")         # slot->block
        st5 = np.concatenate(
            [tails[blk_perm, None],
             np.take_along_axis(full, win_perm, axis=1)[blk_perm]], axis=1
        )
        per_core.append(dict(idx_e=idx_e, bw=bw, u=u, cnt=cnt, full=full,
                             rem=rem, win_perm=win_perm, blk_perm=blk_perm,
                             st5=st5))
    return per_core


def make_profiles(tables, cfg=_DEFAULT_CFG):
    """Per-table profile[s] = [tails>=1, win ranks...] max over cores."""
    profs = []
    for per_core in tables:
        st = np.stack([pc["st5"] for pc in per_core]).max(axis=0)
        st[:, 0] = np.maximum(st[:, 0], 1)
        profs.append(st)
    return profs


def _vert_rows(pc, cfg):
    """vert[s, vin]: local vertex id at psum row vin of slot s (may be >= VLOC)."""
    blk_perm, win_perm = pc["blk_perm"], pc["win_perm"]
    w = win_perm[blk_perm]                                   # [NBLK, 4] rank->win
    vin_off = (w[:, :, None] * WIN + np.arange(WIN)).reshape(cfg.NBLK, BLK_V)
    return blk_perm[:, None] * BLK_V + vin_off


def fill_stream(pc, features, lay, cfg=_DEFAULT_CFG):
    """Per-core bf16 stream, partition-major per 7-slot group."""
    prof_t, fo, iw = lay["prof_t"], lay["fo"], lay["iw"]
    fb, fw, ib, lw, gb = lay["fb"], lay["fw"], lay["ib"], lay["lw"], lay["gb"]

    blk_perm, win_perm = pc["blk_perm"], pc["win_perm"]
    inv_blk = np.empty(cfg.NBLK, dtype=np.int64)
    inv_blk[blk_perm] = np.arange(cfg.NBLK)
    winrank = np.empty((cfg.NBLK, NW), dtype=np.int64)
    np.put_along_axis(winrank, win_perm,
                      np.broadcast_to(np.arange(NW), (cfg.NBLK, NW)), axis=1)

    bw, u, cnt, full, rem = pc["bw"], pc["u"], pc["cnt"], pc["full"], pc["rem"]
    starts = np.zeros(cfg.NBLK * NW, dtype=np.int64)
    np.cumsum(cnt.ravel()[:-1], out=starts[1:])
    r = np.arange(len(bw), dtype=np.int64) - starts[bw]
    b = bw >> 2
    w = bw & 3
    s = inv_blk[b]
    j = winrank[b, w]
    # tail offset of window rank j within block b: cumsum of rem in rank order
    rem_rank = np.take_along_axis(rem, win_perm, axis=1)     # [NBLK, 4] by rank
    toff = np.zeros((cfg.NBLK, NW), dtype=np.int64)
    toff[:, 1:] = np.cumsum(rem_rank[:, :-1], axis=1)

    is_full = r < full[b, w] * BLK
    rt = toff[b, j] + (r - full[b, w] * BLK)                 # tail rank (where used)
    i_flat = np.where(is_full, fo[s, j] + (r >> 7), rt >> 7)
    p = np.where(is_full, r & 127, rt & 127)
    # one-hot position (bf16): windows vertex-in-window, tails vertex-in-block
    pos = np.where(is_full, u, j * WIN + u).astype(ml_dtypes.bfloat16)

    hi = features.astype(ml_dtypes.bfloat16).view(np.uint16)

    rb = np.zeros(cfg.NBLK + 1, dtype=np.int64)
    rb[1:] = np.cumsum(prof_t * BLK)
    rows = np.zeros((int(rb[-1]), F), dtype=np.uint16)
    rowid = rb[s] + (i_flat << 7) + p
    rows[rowid] = hi[pc["idx_e"]]

    # idx store; default 0 -> padding rows one-hot col 0 with zero features
    ival = np.zeros(int(rb[-1]), dtype=ml_dtypes.bfloat16)
    ival[rowid] = pos
    ival_u = ival.view(np.uint16)

    stream = np.empty(int(gb[-1]), dtype=np.uint16)
    for g in range(cfg.NG):
        vg = stream[int(gb[g]) : int(gb[g + 1])].reshape(BLK, int(lw[g]))
        for q in range(GRP):
            ss = g * GRP + q
            pt = int(prof_t[ss])
            blkrows = rows[rb[ss] : rb[ss + 1]].reshape(pt, BLK, F)
            vg[:, int(fb[g, q]) : int(fb[g, q]) + pt * F] = (
                blkrows.transpose(1, 0, 2).reshape(BLK, pt * F)
            )
            iarr = ival_u[rb[ss] : rb[ss + 1]].reshape(pt, BLK).T  # [128, pt]
            o = int(fw[g]) + int(ib[g, q])
            vg[:, o : o + pt] = iarr
    return stream.view(ml_dtypes.bfloat16)


def prep_adjacency(adjacency, pcs, cfg=_DEFAULT_CFG):
    """adj8[c][g, vin, q*T*N + t*N + n] for the permuted vertex at (slot, vin)."""
    adj = np.ascontiguousarray(adjacency.reshape(cfg.V, T, N)).astype(np.int8)
    outs = []
    for c in range(cfg.NCORES):
        apad = np.full((cfg.VPAD, T, N), -1, dtype=np.int8)
        lo = c * cfg.VLOC
        apad[: cfg.VLOC] = adj[lo : lo + cfg.VLOC]
        dev = np.empty((cfg.NBLK, BLK_V, T, N), dtype=np.int8)
        for t in range(T):
            vert = _vert_rows(pcs[t][c], cfg)           # [NBLK, 128]
            dev[:, :, t, :] = apad[vert, t, :]
        dev = dev.reshape(cfg.NG, GRP, BLK_V, T * N).transpose(0, 2, 1, 3)
        outs.append(np.ascontiguousarray(dev).reshape(cfg.NG, BLK, GRP * T * N))
    return outs


def prepare_inputs(adjacency, indices0, features0, indices1, features1,
                   cfg=_DEFAULT_CFG):
    adjacency = np.asarray(adjacency)
    pcs = [shard_table(np.asarray(indices0), cfg),
           shard_table(np.asarray(indices1), cfg)]
    profiles = make_profiles(pcs, cfg)
    lays = [_layout(p) for p in profiles]

    feats = [np.asarray(features0, dtype=np.float32),
             np.asarray(features1, dtype=np.float32)]
    adj8 = prep_adjacency(adjacency, pcs, cfg)
    tmax_t = max(l["tmax_t"] for l in lays)
    tmax_w = max(l["tmax_w"] for l in lays)
    iota = np.broadcast_to(
        np.concatenate([np.tile(np.arange(BLK_V), tmax_t),
                        np.tile(np.arange(WIN), tmax_w)]
                       ).astype(ml_dtypes.bfloat16),
        (BLK, tmax_t * BLK_V + tmax_w * WIN),
    ).copy()

    in_maps = []
    for c in range(cfg.NCORES):
        m = {"adj": adj8[c], "iota": iota}
        for t in range(T):
            m[f"feat{t}"] = fill_stream(pcs[t][c], feats[t], lays[t], cfg)
        in_maps.append(m)
    return in_maps, profiles, pcs


def assemble_output(core_outs, pcs, cfg=_DEFAULT_CFG):
    outs = []
    for t in range(T):
        parts = []
        for c in range(cfg.NCORES):
            res = np.asarray(core_outs[c], dtype=np.float32).reshape(
                cfg.NG, BLK, GRP, T, F
            )
            sres = res.transpose(0, 2, 1, 3, 4).reshape(cfg.NBLK, BLK, T, F)
            vert = _vert_rows(pcs[t][c], cfg)
            full = np.empty((cfg.VPAD, F), dtype=np.float32)
            full[vert.ravel()] = sres[:, :, t, :].reshape(-1, F)
            parts.append(full[: cfg.VLOC])
        outs.append(np.concatenate(parts, axis=0).reshape(B, cfg.V, F))
    return (outs[0], outs[1])


def kernel(adjacency, indices0, features0, indices1, features1):
    from concourse.bass_utils import run_bass_kernel_spmd

    cfg = _DEFAULT_CFG
    in_maps, profiles, pcs = prepare_inputs(
        adjacency, indices0, features0, indices1, features1, cfg
    )

    key = b"".join(p.tobytes() for p in profiles)
    if key not in _NC_CACHE:
        _NC_CACHE[key] = build_device_program(profiles, cfg)
    nc = _NC_CACHE[key]

    res = run_bass_kernel_spmd(nc, in_maps, list(range(cfg.NCORES)))
    return assemble_output(
        [res.results[c]["out"] for c in range(cfg.NCORES)], pcs, cfg
    )


# revision 34
# speedup vs baseline: 1.4302x; 1.4302x over previous
"""Trainium2 Bass kernel for GNN mean aggregation (nn_AggrGSMean).

Computes, for t in {0,1}:
    out_t[b, v, :] = segment_sum(features_t over edges with dest v) / degree[b, v, t]
where degree[b, v, t] = max(count(adjacency[b, v, t, :] >= 0), 1).

Strategy (graph-partition sharding):
- Host: partition edges by destination-vertex range across 8 cores, sort each
  core's edges by destination.  Vertices form 128-wide blocks (the PSUM
  partition dim), each split into four 32-vertex windows.  Full 128-edge
  tiles are window-pure (32-column one-hot via PE tile_position); each
  window's leftover edges pool into block-level "tail" tiles with 128-wide
  one-hots, so roundup waste is per-block not per-window.  Windows are
  sorted within a block and blocks sorted by tile count; a per-table static
  profile (max over cores at each rank) serves all cores.  Features ship as
  single bf16 (the 2e-2 rel-err budget gives ~10x margin); each edge carries
  a bf16 one-hot position (vertex-in-window for full tiles, vertex-in-block
  for tails; both exact in bf16).
- The HBM stream is partition-major per 7-slot group, so each DMA moves
  [128, ~16KB] with fully contiguous lines.  Table 0 streams on the
  sync-engine HWDGE queue, table 1 on the scalar-engine queue.
- Device per (slot, table): GPSIMD builds the tail one-hots, DVE the window
  one-hots (is_equal vs tiled iota); matmuls accumulate into one [128, 64]
  PSUM tile (tails full-width first with start=True, then 32-column windows
  at tile_position (0, 32j)).  Degree comes from an int8 adjacency slice
  (is_ge + reduce + recip on DVE); the mean division rides a ScalarE copy
  into a grouped bf16 output tile.
"""

import sys

if "/opt/trn_rl_repo" not in sys.path:
    sys.path.insert(0, "/opt/trn_rl_repo")

import ml_dtypes
import numpy as np

# Problem constants (hardcoded per contract)
B, V, T, N, F, M = 1, 100000, 2, 32, 64, 1600000
NCORES = 8
BLK = 128            # edges per tile (matmul contraction)
BLK_V = 128          # vertices per block (PSUM partition dim)
WIN = 32             # vertices per one-hot window (stationary columns)
NW = BLK_V // WIN    # windows per block
GRP = 7              # slots per DMA group
USE_TAILS = False    # pool window remainders into 128-wide tail tiles


class Cfg:
    def __init__(self, v=V, ncores=NCORES):
        self.V = v
        self.NCORES = ncores
        self.VLOC = v // ncores
        nblk = (self.VLOC + BLK_V - 1) // BLK_V
        self.NBLK = ((nblk + GRP - 1) // GRP) * GRP
        self.NG = self.NBLK // GRP
        self.VPAD = self.NBLK * BLK_V


_DEFAULT_CFG = Cfg()
_NC_CACHE = {}


def _layout(prof5):
    """Derived layout from a [NBLK, 5] profile (col 0 tails, 1..4 windows)."""
    prof5 = np.asarray(prof5, dtype=np.int64)
    nblk = prof5.shape[0]
    tails = prof5[:, 0]
    wins = prof5[:, 1:]
    prof_t = prof5.sum(axis=1)                        # feat tiles per slot
    fo = np.zeros((nblk, NW), dtype=np.int64)         # flat tile offset of window rank
    fo[:, 0] = tails
    fo[:, 1:] = tails[:, None] + np.cumsum(wins[:, :-1], axis=1)
    ohc = tails * BLK_V + wins.sum(axis=1) * WIN      # one-hot cols per slot
    iw = prof_t                                       # idx cols per slot (bf16 pos)
    ng = nblk // GRP
    pt = prof_t.reshape(ng, GRP)
    iwg = iw.reshape(ng, GRP)
    fb = np.zeros((ng, GRP), dtype=np.int64)          # feat word offset in group line
    fb[:, 1:] = np.cumsum(pt[:, :-1] * F, axis=1)
    fw = (pt * F).sum(axis=1)                         # feat words per group line
    ib = np.zeros((ng, GRP), dtype=np.int64)
    ib[:, 1:] = np.cumsum(iwg[:, :-1], axis=1)
    lw = fw + iwg.sum(axis=1)                         # words per partition line
    gb = np.zeros(ng + 1, dtype=np.int64)
    gb[1:] = np.cumsum(lw * BLK)
    return dict(prof5=prof5, tails=tails, wins=wins, prof_t=prof_t, fo=fo,
                ohc=ohc, iw=iw, fb=fb, fw=fw, ib=ib, lw=lw, gb=gb,
                tmax_t=int(tails.max()), tmax_w=int(wins.sum(axis=1).max()),
                ohmax=int(ohc.max()))


def build_device_program(profiles, cfg=_DEFAULT_CFG):
    """Build + compile the per-core Bass program for per-table [NBLK,5] profiles."""
    from contextlib import ExitStack

    import concourse.tile as tile
    from concourse import bacc, mybir

    f32 = mybir.dt.float32
    bf16 = mybir.dt.bfloat16
    i16 = mybir.dt.int16
    i8 = mybir.dt.int8

    lays = [_layout(p) for p in profiles]
    NBLK, NG = cfg.NBLK, cfg.NG
    tmax_t = max(l["tmax_t"] for l in lays)
    tmax_w = max(l["tmax_w"] for l in lays)
    ohmax = max(l["ohmax"] for l in lays)
    lwmax = max(int(l["lw"].max()) for l in lays)

    nc = bacc.Bacc("TRN2", target_bir_lowering=False, debug=False)
    feat_d = [
        nc.dram_tensor(f"feat{t}", [int(lays[t]["gb"][-1])], bf16,
                       kind="ExternalInput").ap()
        for t in range(T)
    ]
    adj_d = nc.dram_tensor("adj", [NG, BLK, GRP * T * N], i8, kind="ExternalInput").ap()
    iota_d = nc.dram_tensor("iota", [BLK, tmax_t * BLK_V + tmax_w * WIN], bf16,
                            kind="ExternalInput").ap()
    out_d = nc.dram_tensor("out", [NG, BLK, GRP * T * F], bf16,
                           kind="ExternalOutput").ap()

    with tile.TileContext(nc) as tc, ExitStack() as ctx:
        const = ctx.enter_context(tc.tile_pool(name="const", bufs=1))
        featp = ctx.enter_context(tc.tile_pool(name="featp", bufs=4))
        adjp = ctx.enter_context(tc.tile_pool(name="adjp", bufs=3))
        degp = ctx.enter_context(tc.tile_pool(name="degp", bufs=3))
        ohp = ctx.enter_context(tc.tile_pool(name="ohp", bufs=6))
        outp = ctx.enter_context(tc.tile_pool(name="outp", bufs=3))
        psump = ctx.enter_context(tc.tile_pool(name="psum", bufs=8, space="PSUM"))

        iota = const.tile([BLK, tmax_t * BLK_V + tmax_w * WIN], bf16)
        nc.sync.dma_start(out=iota[:], in_=iota_d[:])
        wio = tmax_t * BLK_V  # window iota offset

        for g in range(NG):
            adj_t = adjp.tile([BLK, GRP * T * N], i8)
            nc.sync.dma_start(out=adj_t[:], in_=adj_d[g])
            val = degp.tile([BLK, GRP * T * N], bf16, tag="val")
            nc.vector.tensor_scalar(
                val[:], adj_t[:], 0, None, op0=mybir.AluOpType.is_ge
            )
            deg = degp.tile([BLK, GRP * T], f32, tag="deg")
            nc.vector.tensor_reduce(
                deg[:],
                val[:].rearrange("p (x n) -> p x n", n=N),
                axis=mybir.AxisListType.X,
                op=mybir.AluOpType.add,
            )
            rec = degp.tile([BLK, GRP * T], f32, tag="rec")
            nc.vector.tensor_scalar(
                deg[:], deg[:], 1.0, None, op0=mybir.AluOpType.max
            )
            nc.vector.reciprocal(rec[:], deg[:])

            feats = []
            for t in range(T):
                lay = lays[t]
                ft = featp.tile([BLK, lwmax], bf16, tag=f"feat{t}")
                src = feat_d[t][int(lay["gb"][g]) : int(lay["gb"][g + 1])].rearrange(
                    "(p w) -> p w", w=int(lay["lw"][g])
                )
                eng = nc.sync if t == 0 else nc.scalar
                eng.dma_start(out=ft[:, : int(lay["lw"][g])], in_=src)
                feats.append(ft)

            out_t = outp.tile([BLK, GRP * T * F], bf16)
            for q in range(GRP):
                s = g * GRP + q
                for t in range(T):
                    lay = lays[t]
                    ft = feats[t]
                    nt = int(lay["tails"][s])
                    nwt = int(lay["wins"][s].sum())
                    ts = nt + nwt
                    fbase = int(lay["fb"][g, q])
                    io = int(lay["fw"][g]) + int(lay["ib"][g, q])
                    oh = ohp.tile([BLK, ohmax], bf16, tag="oh")
                    # tail one-hots (128-wide) on DVE
                    if nt > 0:
                        nc.vector.tensor_tensor(
                            oh[:, : nt * BLK_V].rearrange(
                                "p (i v) -> p i v", v=BLK_V
                            ),
                            iota[:, : nt * BLK_V].rearrange(
                                "p (i v) -> p i v", v=BLK_V
                            ),
                            ft[:, io : io + nt].unsqueeze(2)
                            .broadcast_to([BLK, nt, BLK_V]),
                            op=mybir.AluOpType.is_equal,
                        )
                    # window one-hots (32-wide) on DVE
                    if nwt > 0:
                        nc.vector.tensor_tensor(
                            oh[:, nt * BLK_V : nt * BLK_V + nwt * WIN].rearrange(
                                "p (i v) -> p i v", v=WIN
                            ),
                            iota[:, wio : wio + nwt * WIN].rearrange(
                                "p (i v) -> p i v", v=WIN
                            ),
                            ft[:, io + nt : io + ts].unsqueeze(2)
                            .broadcast_to([BLK, nwt, WIN]),
                            op=mybir.AluOpType.is_equal,
                        )
                    ps = psump.tile([BLK, F], f32)
                    for i in range(nt):
                        nc.tensor.matmul(
                            ps[:],
                            lhsT=oh[:, i * BLK_V : (i + 1) * BLK_V],
                            rhs=ft[:, fbase + i * F : fbase + (i + 1) * F],
                            start=(i == 0),
                            stop=(i == ts - 1),
                        )
                    wco = nt * BLK_V
                    for j in range(NW):
                        nj = int(lay["wins"][s, j])
                        for il in range(nj):
                            i = int(lay["fo"][s, j]) + il
                            c = wco + (i - nt) * WIN
                            nc.tensor.matmul(
                                ps[j * WIN : (j + 1) * WIN, :],
                                lhsT=oh[:, c : c + WIN],
                                rhs=ft[:, fbase + i * F : fbase + (i + 1) * F],
                                start=(nt == 0 and il == 0),
                                stop=(i == ts - 1) if nt else (il == nj - 1),
                                tile_position=(0, j * WIN),
                            )
                    nc.scalar.mul(
                        out_t[:, (q * T + t) * F : (q * T + t + 1) * F],
                        ps[:],
                        rec[:, q * T + t : q * T + t + 1],
                    )
            nc.sync.dma_start(out=out_d[g], in_=out_t[:])

    nc.compile()
    return nc


def shard_table(indices, cfg=_DEFAULT_CFG):
    """Sort edges by destination, partition by core, build per-core schedule."""
    v = np.ascontiguousarray(indices[:, 1])
    order = np.argsort(v, kind="stable")
    vs = v[order]
    bounds = np.searchsorted(vs, np.arange(cfg.NCORES + 1) * cfg.VLOC)
    per_core = []
    for c in range(cfg.NCORES):
        lo, hi = bounds[c], bounds[c + 1]
        idx_e = order[lo:hi]
        vloc = vs[lo:hi].astype(np.int64) - c * cfg.VLOC
        bw = vloc >> 5                       # block*4 + window
        u = vloc & 31
        cnt = np.bincount(bw, minlength=cfg.NBLK * NW).reshape(cfg.NBLK, NW)
        if USE_TAILS:
            full = cnt >> 7
            rem = cnt & 127
            tails = (rem.sum(axis=1) + BLK - 1) // BLK
        else:
            full = (cnt + BLK - 1) >> 7
            rem = np.zeros_like(cnt)
            tails = np.zeros(cfg.NBLK, dtype=np.int64)
        win_perm = np.argsort(-full, axis=1, kind="stable")    # [NBLK, 4] rank->win
        blk_tot = tails + full.sum(axis=1)
        blk_perm = np.argsort(-blk_tot, kind="stable")         # slot->block
        st5 = np.concatenate(
            [tails[blk_perm, None],
             np.take_along_axis(full, win_perm, axis=1)[blk_perm]], axis=1
        )
        per_core.append(dict(idx_e=idx_e, bw=bw, u=u, cnt=cnt, full=full,
                             rem=rem, win_perm=win_perm, blk_perm=blk_perm,
                             st5=st5))
    return per_core


def make_profiles(tables, cfg=_DEFAULT_CFG):
    """Per-table profile[s] = [tails>=1, win ranks...] max over cores."""
    profs = []
    for per_core in tables:
        st = np.stack([pc["st5"] for pc in per_core]).max(axis=0)
        if USE_TAILS:
            st[:, 0] = np.maximum(st[:, 0], 1)
        else:
            st[:, 1:] = np.maximum(st[:, 1:], 1)
        profs.append(st)
    return profs


def _vert_rows(pc, cfg):
    """vert[s, vin]: local vertex id at psum row vin of slot s (may be >= VLOC)."""
    blk_perm, win_perm = pc["blk_perm"], pc["win_perm"]
    w = win_perm[blk_perm]                                   # [NBLK, 4] rank->win
    vin_off = (w[:, :, None] * WIN + np.arange(WIN)).reshape(cfg.NBLK, BLK_V)
    return blk_perm[:, None] * BLK_V + vin_off


def fill_stream(pc, features, lay, cfg=_DEFAULT_CFG):
    """Per-core bf16 stream, partition-major per 7-slot group."""
    prof_t, fo, iw = lay["prof_t"], lay["fo"], lay["iw"]
    fb, fw, ib, lw, gb = lay["fb"], lay["fw"], lay["ib"], lay["lw"], lay["gb"]

    blk_perm, win_perm = pc["blk_perm"], pc["win_perm"]
    inv_blk = np.empty(cfg.NBLK, dtype=np.int64)
    inv_blk[blk_perm] = np.arange(cfg.NBLK)
    winrank = np.empty((cfg.NBLK, NW), dtype=np.int64)
    np.put_along_axis(winrank, win_perm,
                      np.broadcast_to(np.arange(NW), (cfg.NBLK, NW)), axis=1)

    bw, u, cnt, full, rem = pc["bw"], pc["u"], pc["cnt"], pc["full"], pc["rem"]
    starts = np.zeros(cfg.NBLK * NW, dtype=np.int64)
    np.cumsum(cnt.ravel()[:-1], out=starts[1:])
    r = np.arange(len(bw), dtype=np.int64) - starts[bw]
    b = bw >> 2
    w = bw & 3
    s = inv_blk[b]
    j = winrank[b, w]
    # tail offset of window rank j within block b: cumsum of rem in rank order
    rem_rank = np.take_along_axis(rem, win_perm, axis=1)     # [NBLK, 4] by rank
    toff = np.zeros((cfg.NBLK, NW), dtype=np.int64)
    toff[:, 1:] = np.cumsum(rem_rank[:, :-1], axis=1)

    is_full = r < full[b, w] * BLK
    rt = toff[b, j] + (r - full[b, w] * BLK)                 # tail rank (where used)
    i_flat = np.where(is_full, fo[s, j] + (r >> 7), rt >> 7)
    p = np.where(is_full, r & 127, rt & 127)
    # one-hot position (bf16): windows vertex-in-window, tails vertex-in-block
    pos = np.where(is_full, u, j * WIN + u).astype(ml_dtypes.bfloat16)

    hi = features.astype(ml_dtypes.bfloat16).view(np.uint16)

    rb = np.zeros(cfg.NBLK + 1, dtype=np.int64)
    rb[1:] = np.cumsum(prof_t * BLK)
    rows = np.zeros((int(rb[-1]), F), dtype=np.uint16)
    rowid = rb[s] + (i_flat << 7) + p
    rows[rowid] = hi[pc["idx_e"]]

    # idx store; default 0 -> padding rows one-hot col 0 with zero features
    ival = np.zeros(int(rb[-1]), dtype=ml_dtypes.bfloat16)
    ival[rowid] = pos
    ival_u = ival.view(np.uint16)

    stream = np.empty(int(gb[-1]), dtype=np.uint16)
    for g in range(cfg.NG):
        vg = stream[int(gb[g]) : int(gb[g + 1])].reshape(BLK, int(lw[g]))
        for q in range(GRP):
            ss = g * GRP + q
            pt = int(prof_t[ss])
            blkrows = rows[rb[ss] : rb[ss + 1]].reshape(pt, BLK, F)
            vg[:, int(fb[g, q]) : int(fb[g, q]) + pt * F] = (
                blkrows.transpose(1, 0, 2).reshape(BLK, pt * F)
            )
            iarr = ival_u[rb[ss] : rb[ss + 1]].reshape(pt, BLK).T  # [128, pt]
            o = int(fw[g]) + int(ib[g, q])
            vg[:, o : o + pt] = iarr
    return stream.view(ml_dtypes.bfloat16)


def prep_adjacency(adjacency, pcs, cfg=_DEFAULT_CFG):
    """adj8[c][g, vin, q*T*N + t*N + n] for the permuted vertex at (slot, vin)."""
    adj = np.ascontiguousarray(adjacency.reshape(cfg.V, T, N)).astype(np.int8)
    outs = []
    for c in range(cfg.NCORES):
        apad = np.full((cfg.VPAD, T, N), -1, dtype=np.int8)
        lo = c * cfg.VLOC
        apad[: cfg.VLOC] = adj[lo : lo + cfg.VLOC]
        dev = np.empty((cfg.NBLK, BLK_V, T, N), dtype=np.int8)
        for t in range(T):
            vert = _vert_rows(pcs[t][c], cfg)           # [NBLK, 128]
            dev[:, :, t, :] = apad[vert, t, :]
        dev = dev.reshape(cfg.NG, GRP, BLK_V, T * N).transpose(0, 2, 1, 3)
        outs.append(np.ascontiguousarray(dev).reshape(cfg.NG, BLK, GRP * T * N))
    return outs


def prepare_inputs(adjacency, indices0, features0, indices1, features1,
                   cfg=_DEFAULT_CFG):
    adjacency = np.asarray(adjacency)
    pcs = [shard_table(np.asarray(indices0), cfg),
           shard_table(np.asarray(indices1), cfg)]
    profiles = make_profiles(pcs, cfg)
    lays = [_layout(p) for p in profiles]

    feats = [np.asarray(features0, dtype=np.float32),
             np.asarray(features1, dtype=np.float32)]
    adj8 = prep_adjacency(adjacency, pcs, cfg)
    tmax_t = max(l["tmax_t"] for l in lays)
    tmax_w = max(l["tmax_w"] for l in lays)
    iota = np.broadcast_to(
        np.concatenate([np.tile(np.arange(BLK_V), tmax_t),
                        np.tile(np.arange(WIN), tmax_w)]
                       ).astype(ml_dtypes.bfloat16),
        (BLK, tmax_t * BLK_V + tmax_w * WIN),
    ).copy()

    in_maps = []
    for c in range(cfg.NCORES):
        m = {"adj": adj8[c], "iota": iota}
        for t in range(T):
            m[f"feat{t}"] = fill_stream(pcs[t][c], feats[t], lays[t], cfg)
        in_maps.append(m)
    return in_maps, profiles, pcs


def assemble_output(core_outs, pcs, cfg=_DEFAULT_CFG):
    outs = []
    for t in range(T):
        parts = []
        for c in range(cfg.NCORES):
            res = np.asarray(core_outs[c], dtype=np.float32).reshape(
                cfg.NG, BLK, GRP, T, F
            )
            sres = res.transpose(0, 2, 1, 3, 4).reshape(cfg.NBLK, BLK, T, F)
            vert = _vert_rows(pcs[t][c], cfg)
            full = np.empty((cfg.VPAD, F), dtype=np.float32)
            full[vert.ravel()] = sres[:, :, t, :].reshape(-1, F)
            parts.append(full[: cfg.VLOC])
        outs.append(np.concatenate(parts, axis=0).reshape(B, cfg.V, F))
    return (outs[0], outs[1])


def kernel(adjacency, indices0, features0, indices1, features1):
    from concourse.bass_utils import run_bass_kernel_spmd

    cfg = _DEFAULT_CFG
    in_maps, profiles, pcs = prepare_inputs(
        adjacency, indices0, features0, indices1, features1, cfg
    )

    key = b"".join(p.tobytes() for p in profiles)
    if key not in _NC_CACHE:
        _NC_CACHE[key] = build_device_program(profiles, cfg)
    nc = _NC_CACHE[key]

    res = run_bass_kernel_spmd(nc, in_maps, list(range(cfg.NCORES)))
    return assemble_output(
        [res.results[c]["out"] for c in range(cfg.NCORES)], pcs, cfg
    )
